# revision 102
# baseline (speedup 1.0000x reference)
"""CRvNN forward kernel for 8x Trainium2 NeuronCores (Bass/Tile), v4.

Strategy
--------
Pure data parallelism: batch 32 -> 4 per core; params replicated; no
collectives.  State is TRANSPOSED (D=256 on partitions as 2x128 chunks,
position on the free axis, padded 514 -> 516).  The S^2 neighbor-prob
matrices are first-order linear recurrences evaluated as tensor_tensor_
scans; they are never materialized.

v4 design (vs the 647us v3):
- Positions >= 512 (the last real token, END, pads) have transition prob
  identically 0 (selp mask), so their composer/LN/score outputs are never
  used and the sequence state there never changes.  All steady-state
  matmuls, PSUM reads, and composer/apply elementwise ops therefore run
  on columns [0:512) only; the in-place apply leaves the seqT tail
  untouched (it is correct by construction).  Scans/fills stay full
  width (the backward scans read the tail as input).
- PSUM: matmul groups use flat [128,1024] tiles (2 banks, bufs=2) so two
  128-col output chunks share ONE Activation read (1038ns vs 2x615).
  512-wide row broadcasts use a dedicated 1-bank [128,512] tile (bufs=2).
  All batched stats (tsc, LN mean, LN var) pack into a single [12,512]
  bank as per-(section,batch) partition rows with per-batch accumulation
  groups; a tiny [12,4] bank holds init-only tail stats.  8 banks total.
- w2 (1024x1024) runs fp8e4 + MatmulPerfMode.DoubleRow (0.5 cyc/col);
  weights host-prescaled by 64, 1/64 folded into the PSUM-read scale.
- Composer: gate order (parent, g2, g0, g1); the single Pool multiply
  (g2*parent) is issued early and overlaps the remaining gate reads;
  everything else accumulates on DVE at fp16 2x rates.
- LN rows: mean+var read as ONE (8,512) Activation op, rstd via fused
  Rsqrt; 1/D folded into the PE stat selectors.

This walrus build supports only ONE sync wait per instruction; a
post-scheduling pass splits multi-wait instructions into single-wait NOP
chains.
"""
import os
import sys
from contextlib import ExitStack

import numpy as np

sys.path.insert(0, "/opt/trn_rl_repo")

import ml_dtypes
import bass_rust
import concourse.bass as bass
import concourse.mybir as mybir
from concourse.tile import TileContext

F32 = mybir.dt.float32
F16 = mybir.dt.float16
BF16 = mybir.dt.bfloat16
F8 = mybir.dt.float8e4
AL = mybir.AluOpType
AF = mybir.ActivationFunctionType
PM = mybir.MatmulPerfMode

NCORES = 8
NB = 4            # batch per core
D = 256
DC = 2            # D chunks of 128
S2 = 514
SP = 516          # padded sequence length
SW = 512          # steady-state work width (cols >= 512 never transition)
SPP = SP + 2      # scan-input tiles have leading+trailing zero pad columns
NL = 34           # row-tile lanes: batch b lives at partition b%2 + 32*(b//2)
LANES = [0, 1, 32, 33]
H = 1024
WIN = 5
EPS = 1e-9
WSCALE = 64.0     # fp8 weight prescale (folded back via activation scale)

SIM = os.environ.get("CRVNN_SIM", "0") == "1"
TRACE = os.environ.get("CRVNN_TRACE", "0") == "1"
# compat attrs (test.py uses these in its program-cache key)
MM_DT = os.environ.get("CRVNN_MMDT", "f8")
W2_DT = os.environ.get("CRVNN_W2DT", "f8")
GP_LVL = int(os.environ.get("CRVNN_GP", "1"))
# per-matmul-group dtype: "f8" = fp8e4 + DoubleRow; anything else = fp16
F8_W1 = os.environ.get("CRVNN_F8W1", "f16") == "f8"
F8_W2 = os.environ.get("CRVNN_F8W2", "f8") == "f8"
F8_CV = os.environ.get("CRVNN_F8CV", "mixed") == "f8"
MIXED_CV = os.environ.get("CRVNN_F8CV", "mixed") == "mixed"

NSPLITS = [(0, 512), (512, SP - 512)]

LAST_EXEC_NS = None
LAST_RES = None

# engine assignment knobs: 'v' = DVE, 'g' = Pool/gpsimd
ENG = {
    "fill_seq": "g",
    "fill_base": "v",
    "fill_l2": "g",
    "fill_r2": "v",
    "scan_lc": "v",
    "scan_l1": "v",
    "scan_r1": "v",
    "scan_l2": "v",
    "scan_r2": "v",
    "gm": "g",
    "t2": "v",
    "sq": "v",
    "fill_seq2": "g",
    "act_upd": "g",
    "m1": "v",
    "cmul": "v",
}


# --------------------------------------------------------------------------
# post-scheduling fixup: split multi-wait instructions into 1-wait NOP chains
# --------------------------------------------------------------------------
def _split_multiwaits(nc):
    counter = [0]

    def mk_nop(engine, wait):
        counter[0] += 1
        n = bass_rust.InstNoOp(name=f"WFIX-{counter[0]}", ins=[], outs=[])
        n.engine = engine
        n.sync_info = bass_rust.SyncInfo(on_wait=[wait], on_update=[])
        return n

    total = 0
    for f in nc.m.functions:
        for bb in f.blocks:
            out = []
            changed = False
            for inst in list(bb.instructions):
                si = inst.sync_info
                waits = list(si.on_wait) if (si is not None and si.on_wait) else []
                if len(waits) > 1:
                    for w in waits[:-1]:
                        out.append(mk_nop(inst.engine, w))
                    inst.sync_info = bass_rust.SyncInfo(
                        on_wait=[waits[-1]],
                        on_update=list(si.on_update) if si.on_update else [])
                    changed = True
                    total += 1
                out.append(inst)
            if changed:
                bb.instructions = out
    return total


def _bcast_ap(drow):
    """DRAM row AP (1, n) -> partition-broadcast AP (128, n)."""
    return bass.AP(tensor=drow.tensor, offset=drow.offset,
                   ap=[[0, 128]] + drow.ap[1:])


def _build_program(n_steps, flags):
    nc = bass.Bass()

    W1T = F8 if F8_W1 else F16
    W2T = F8 if F8_W2 else F16
    CVT = F8 if F8_CV else F16            # baseT/l1T/r1T tile dtype
    WCV = F8 if F8_CV else F16            # main conv weights dtype
    w1_scale = 1.0 / WSCALE if F8_W1 else 1.0
    w2_scale = 1.0 / WSCALE if F8_W2 else 1.0
    cv_scale = 1.0 / WSCALE if (F8_CV or MIXED_CV) else 1.0

    seqT_in = nc.declare_dram_parameter("seqT", [NB, DC, 128, SP], F16, isOutput=False)
    mask_in = nc.declare_dram_parameter("mask", [NL, SP], F16, isOutput=False)
    selp_in = nc.declare_dram_parameter("selp", [NL, SP], F16, isOutput=False)
    act0_in = nc.declare_dram_parameter("act0", [NB, SP], F16, isOutput=False)
    act0f_in = nc.declare_dram_parameter("act0f", [NL, SP], F32, isOutput=False)
    nact0_in = nc.declare_dram_parameter("nact0", [NB, SP], F16, isOutput=False)
    itW_in = nc.declare_dram_parameter("itW", [D, D], F16, isOutput=False)
    convW_in = nc.declare_dram_parameter("convW", [WIN * D, D], WCV, isOutput=False)
    if MIXED_CV:
        convW8_in = nc.declare_dram_parameter("convW8", [2 * D, D], F8,
                                              isOutput=False)
    w1W_in = nc.declare_dram_parameter("w1W", [2 * D, H], W1T, isOutput=False)
    w2W_in = nc.declare_dram_parameter("w2W", [H, 4 * D], W2T, isOutput=False)
    sc4_in = nc.declare_dram_parameter("sc4", [128, NB, DC, NL], F16, isOutput=False)
    obD_in = nc.declare_dram_parameter("obD", [128, NB, 2], F16, isOutput=False)
    bsel_in = nc.declare_dram_parameter("bsel", [NL, NB, 128], F16, isOutput=False)
    noc_in = nc.declare_dram_parameter("noc", [128, DC], F32, isOutput=False)
    ymn_in = nc.declare_dram_parameter("ymnc", [128, DC], F32, isOutput=False)
    opt_in = {}
    for nm, shape in [("itbc", [128, DC]), ("convbc", [128, DC]),
                      ("w1bc", [128, 8]), ("w2bc", [128, 8]), ("scbc", [4, 1]),
                      ("lngc", [128, DC]), ("lnbc", [128, DC])]:
        if flags.get(nm):
            opt_in[nm] = nc.declare_dram_parameter(nm, shape, F32, isOutput=False)
    out_dram = nc.declare_dram_parameter("out", [NB, DC, 128, S2], F16, isOutput=True)

    CV2 = F8 if (F8_CV or MIXED_CV) else F16

    with TileContext(nc) as tc, ExitStack() as ctx:
        wpool = ctx.enter_context(tc.tile_pool(name="wpool", bufs=1))
        state = ctx.enter_context(tc.tile_pool(name="state", bufs=1))
        work = ctx.enter_context(tc.tile_pool(name="work", bufs=1))
        psum = ctx.enter_context(tc.tile_pool(name="psum", bufs=1, space="PSUM"))
        dram = ctx.enter_context(tc.tile_pool(name="dramp", bufs=1, space="DRAM"))

        V = nc.vector
        G = nc.gpsimd

        def eng(key):
            return G if ENG[key] == "g" else V

        # ---------------- weights -> SBUF ----------------------------------
        # pair tiles: (128, 2, M); [:, i, :] = rows [p*256 + i*128 : +128]
        # (host already converted to the matmul dtype)
        _ldq = [nc.sync, nc.scalar]
        _ldi = [0]

        def ld_dma(out, in_):
            # rotate init loads across the three HWDGE queues so the big
            # weight transfers overlap instead of serializing on SP
            _ldq[_ldi[0] % 2].dma_start(out=out, in_=in_)
            _ldi[0] += 1

        def load_pairs(name, dram_p, n_pairs, M, dt):
            tiles = []
            for p in range(n_pairs):
                t = wpool.tile([128, 2, M], dt, name=f"{name}{p}")
                ld_dma(t, dram_p.ap()[p * 256:(p + 1) * 256, :].rearrange(
                    "(two q) m -> q two m", two=2))
                tiles.append(t)
            return tiles

        w1W8 = load_pairs("w1W8", w1W_in, 2, H, W1T)
        w2W8 = load_pairs("w2W8", w2W_in, 4, 4 * D, W2T)
        if MIXED_CV:
            # dtype-matched weights: f16 (x64-prescaled) for pieces 1..3,
            # fp8 for pieces 0 (l2) and 4 (r2).  Skip the unused f16 pairs.
            cvW8 = [None] * 5
            for p in (1, 2, 3):
                t = wpool.tile([128, 2, D], F16, name=f"cvW16_{p}")
                nc.sync.dma_start(
                    out=t,
                    in_=convW_in.ap()[p * 256:(p + 1) * 256, :].rearrange(
                        "(two q) m -> q two m", two=2))
                cvW8[p] = t
            cvW8f8 = load_pairs("cvW8f8", convW8_in, 2, D, F8)
        else:
            cvW8 = load_pairs("cvW8", convW_in, 5, D, WCV)

        itW_t = wpool.tile([128, 2, D], F16, name="itWt")
        ld_dma(itW_t, itW_in.ap().rearrange("(two q) m -> q two m", two=2))
        sc4 = wpool.tile([128, NB, DC, NL], F16, name="sc4t")
        ld_dma(sc4, sc4_in.ap())
        obD = wpool.tile([128, NB, 2], F16, name="obDt")
        ld_dma(obD, obD_in.ap())
        bsel = wpool.tile([NL, NB, 128], F16, name="bselt")
        ld_dma(bsel, bsel_in.ap())
        noc = wpool.tile([128, DC], F32, name="noct")
        ld_dma(noc, noc_in.ap())
        ymnc = wpool.tile([128, DC], F32, name="ymnct")
        ld_dma(ymnc, ymn_in.ap())
        eps4 = wpool.tile([NL, 1], F32, name="eps4")
        nc.vector.memset(eps4, 1e-5)
        one34 = wpool.tile([NL, 1], F32, name="one34")
        nc.vector.memset(one34, 1.0)


        def load_opt(nm, shape):
            if nm not in opt_in:
                return None
            t = wpool.tile(shape, F32, name=f"{nm}_t")
            nc.sync.dma_start(out=t, in_=opt_in[nm].ap())
            return t

        itb_t = load_opt("itbc", [128, DC])
        convb_t = load_opt("convbc", [128, DC])
        w1b_t = load_opt("w1bc", [128, 8])
        w2b_t = load_opt("w2bc", [128, 8])
        scb_t = load_opt("scbc", [4, 1])
        lng_t = load_opt("lngc", [128, DC])
        lnb_t = load_opt("lnbc", [128, DC])
        # the fused-Activation fast paths assume bias-free layers
        fuse_w1 = w1b_t is None
        fuse_w2 = w2b_t is None
        fuse_cv = convb_t is None

        # ---------------- persistent state ---------------------------------
        seqT = [state.tile([128, DC, SP], F16, name=f"seqT{b}") for b in range(NB)]
        if F8_W1:
            seqT8 = [state.tile([128, DC, SP], F8, name=f"seqT8_{b}")
                     for b in range(NB)]
        else:
            seqT8 = seqT
        a4 = state.tile([NL, SP], F32, name="a4")
        ld_dma(a4, act0f_in.ap())
        mask4 = state.tile([NL, SP], F16, name="mask4")
        ld_dma(mask4, mask_in.ap())
        selp4 = state.tile([NL, SP], F16, name="selp4")
        ld_dma(selp4, selp_in.ap())

        # PSUM: 8 banks total.
        #   psmm  [128,1024] x2 bufs  = 4 banks (matmul groups, flat)
        #   psbc  [128, 512] x2 bufs  = 2 banks (512-wide row broadcasts)
        #   ps_big [12, 512] x1       = 1 bank  (tsc b | mean 4+b | var 8+b)
        #   ps_it  [12,   4] x1       = 1 bank  (init-only stat tails)
        def psmm(name):
            return psum.tile([128, 2 * SW], F32, name=name, tag="psmm", bufs=2)

        def psbc_t(name):
            return psum.tile([128, SW], F32, name=name, tag="psbc", bufs=2)

        # sections at base partitions 0 (tsc), 32 (mean), 64 (var) -- PE
        # matmul outs must start at partition 0/32/64; one bank each tile.
        # per-HALF stats: batches {2h,2h+1} -> ps_mvh[h]; mean at partitions
        # [0:2], var at [32:34] of the same bank (distinct group flags), so
        # half 0's LN chain unblocks as soon as batch 1's w2 finishes.
        ps_mvh = [psum.tile([34, SW], F32, name=f"ps_mv{h}", tag=f"psmv{h}",
                            bufs=1) for h in range(2)]

        def row4(name, dt=F32, bufs=4):
            return work.tile([NL, SP], dt, name=name, tag="row4", bufs=bufs)

        def bc_tile(name, tag, bufs, w=SP):
            return work.tile([128, w], F16, name=name, tag=tag, bufs=bufs)

        def bounce_bcast(drow_b, name, tag, bufs=4):
            """(1,SP) slice of a DRAM (4,SP) tile -> (128,SP) bcast tile."""
            t = bc_tile(name, tag=tag, bufs=bufs)
            nc.sync.dma_start(out=t, in_=_bcast_ap(drow_b))
            return t

        def napad(name, src_ap):
            """(128, SPP) bcast tile with zero pads at cols 0, SPP-1."""
            t = work.tile([128, SPP], F16, name=name, tag="nabP", bufs=3)
            nc.vector.memset(t[:, 0:SPP:SPP - 1], 0.0)
            nc.sync.dma_start(out=t[:, 1:SP + 1], in_=src_ap)
            return t

        def pe_bcast(row, b, name, tag, copy_eng, pads=False, bufs=4,
                     width=SW):
            """Broadcast row b of a (4,SP) SBUF tile to (128,width) via PE:
            psum[p,i] = sum_q bsel[q,b,p]*row[q,i] = row[b,i], then one
            engine copy PSUM->SBUF.  width=SW uses the 1-bank psbc tile;
            width=SP uses a flat psmm tile (2 matmuls)."""
            if width == SW:
                ps = psbc_t(f"bc_{name}")
                nc.tensor.matmul(ps, bsel[:, b, :], row[:, 0:SW],
                                 start=True, stop=True)
                src = ps
            else:
                ps = psmm(f"bc_{name}")
                for (o, s) in NSPLITS:
                    nc.tensor.matmul(ps[:, o:o + s], bsel[:, b, :],
                                     row[:, o:o + s], start=True, stop=True)
                src = ps[:, 0:SP]
            if pads:
                t = work.tile([128, SPP], F16, name=name, tag="nabP",
                              bufs=bufs)
                nc.vector.memset(t[:, 0:SPP:SPP - 1], 0.0)
                dst = t[:, 1:width + 1]
            else:
                t = bc_tile(name, tag=tag, bufs=bufs, w=width)
                dst = t
            if copy_eng == "act":
                nc.scalar.activation(out=dst, in_=src, func=AF.Copy)
            elif copy_eng == "pool":
                nc.gpsimd.tensor_scalar(out=dst, in0=src, scalar1=1.0,
                                        scalar2=None, op0=AL.mult)
            else:
                nc.vector.tensor_scalar(out=dst, in0=src, scalar1=1.0,
                                        scalar2=None, op0=AL.mult)
            return t

        def pe_bcast_full2(row34, b, name, tag):
            """Full-width (516) broadcast of batch b's lane."""
            h = b // 2
            sl = slice(32 * h, 32 * h + 2)
            ps = psmm(f"bc_{name}")
            for (o, s_) in NSPLITS:
                nc.tensor.matmul(ps[:, o:o + s_], bsel[sl, b, :],
                                 row34[sl, o:o + s_], start=True, stop=True)
            t = bc_tile(name, tag=tag, bufs=2, w=SP)
            nc.scalar.activation(out=t, in_=ps[:, 0:SP], func=AF.Copy)
            return t

        def pe_bcast2(row34, b, name, tag, copy_eng):
            """Broadcast lane of batch b from a 34-lane row tile to
            (128,512) via PE; lhsT slice shares the row slice's base."""
            h = b // 2
            sl = slice(32 * h, 32 * h + 2)
            ps = psbc_t(f"bc_{name}")
            nc.tensor.matmul(ps, bsel[sl, b, :], row34[sl, 0:SW],
                             start=True, stop=True)
            t = bc_tile(name, tag=tag, bufs=4, w=SW)
            if copy_eng == "act":
                nc.scalar.activation(out=t, in_=ps, func=AF.Copy)
            elif copy_eng == "pool":
                nc.gpsimd.tensor_scalar(out=t, in0=ps, scalar1=1.0,
                                        scalar2=None, op0=AL.mult)
            else:
                nc.vector.tensor_scalar(out=t, in0=ps, scalar1=1.0,
                                        scalar2=None, op0=AL.mult)
            return t

        def ax_tile(name, tag):
            t = work.tile([128, DC, SPP], F16, name=name, tag=tag,
                          bufs=(4 if tag == "axs" else 2))
            for c in range(DC):
                nc.vector.memset(t[:, c, 0:SPP:SPP - 1], 0.0)
            return t

        def scan_fwd(e, out_c, nap, datap):
            """out[i] = data[i-1] + na[i-1]*out[i-1]; data pad supplies z0=0."""
            e.tensor_tensor_scan(
                out=out_c, data0=nap[:, 0:SP], data1=datap[:, 0:SP],
                initial=0.0, op0=AL.mult, op1=AL.add)

        def scan_bwd(e, out_c, nap, datap):
            e.tensor_tensor_scan(
                out=out_c[:, ::-1], data0=nap[:, SPP - 1:1:-1],
                data1=datap[:, SPP - 1:1:-1], initial=0.0,
                op0=AL.mult, op1=AL.add)

        def gelu_act(out, in_, bias, scale=1.0, n=SW, p=128):
            b = bias if bias is not None else 0.0
            if SIM:
                x2 = work.tile([p, n], F16, name="gx2", tag="gelu_tmp", bufs=1)
                nc.scalar.activation(out=x2, in_=in_, func=AF.Square, bias=b,
                                     scale=scale)
                nc.vector.tensor_scalar(out=x2, in0=x2, scalar1=0.044715,
                                        scalar2=1.0, op0=AL.mult, op1=AL.add)
                u = work.tile([p, n], F16, name="gu", tag="gelu_tmp2", bufs=1)
                nc.scalar.activation(out=u, in_=in_, func=AF.Identity, bias=b,
                                     scale=scale)
                nc.vector.tensor_tensor(out=x2, in0=x2, in1=u, op=AL.mult)
                nc.scalar.activation(out=x2, in_=x2, func=AF.Tanh,
                                     scale=0.7978845608028654)
                nc.vector.tensor_scalar(out=x2, in0=x2, scalar1=1.0,
                                        scalar2=0.5, op0=AL.add, op1=AL.mult)
                nc.vector.tensor_tensor(out=out, in0=x2, in1=u, op=AL.mult)
            else:
                nc.scalar.activation(out=out, in_=in_, func=AF.Gelu_apprx_tanh,
                                     bias=b, scale=scale)

        # matmul helper: lhsT pair tiles, rhs (128, 2, s) slices
        def mmdr(ps_ap, pairs, f8, nsl=((0, SW),)):
            K = len(pairs)
            for (o, s) in nsl:
                for k, (lhsT, rhs) in enumerate(pairs):
                    if f8:
                        nc.tensor.matmul(ps_ap[:, o:o + s], lhsT,
                                         rhs[:, :, o:o + s],
                                         start=(k == 0), stop=(k == K - 1),
                                         perf_mode=PM.DoubleRow)
                    else:
                        for i in range(2):
                            nc.tensor.matmul(ps_ap[:, o:o + s], lhsT[:, i, :],
                                             rhs[:, i, o:o + s],
                                             start=(k == 0 and i == 0),
                                             stop=(k == K - 1 and i == 1))

        # per-half stat matmuls; one-hot lhsT delivers batch b to partition
        # b%2 of half b//2.  sect 1 = mean (base 0), sect 2 = var (base 32).
        # start/stop are per (half, section) accumulation groups.
        def mm_stat(sect, b, c, rhs, init_tail=None, tail_ps=None,
                    tail_start=False, tail_stop=False):
            p = 32 * (sect - 1)
            nc.tensor.matmul(ps_mvh[b // 2][p:p + 2, :], obD[:, b, :],
                             rhs, start=(b % 2 == 0 and c == 0),
                             stop=(b % 2 == 1 and c == 1))
            if init_tail is not None:
                nc.tensor.matmul(tail_ps[:, 4 * sect:4 * sect + 4],
                                 obD[:, b, :], init_tail,
                                 start=tail_start, stop=tail_stop)

        # ------------------------------------------------------------------
        # LN rows: one (8,512) read of mean|var, rstd via fused Rsqrt.
        # mid-run tails are never used (tpm==0 there): memset to benign 1.0.
        # ------------------------------------------------------------------
        def ln_rows_half(h, tiles, init_tail_ps=None):
            """LN row chain for batch pair {2h,2h+1}: lane slice [32h:32h+2]
            of shared 34-lane tiles, so every op's SBUF operands share a
            32-aligned base partition (BIR verifier requirement).  Rows are
            f16: values are O(1) and the 2e-2 budget tolerates 1e-3 here;
            f16 doubles the DVE rate on this latency-critical chain."""
            mv, msq, vv, rstd = tiles
            sl = slice(32 * h, 32 * h + 2)
            nc.scalar.activation(out=mv[sl, 0, 0:SW], in_=ps_mvh[h][0:2, :],
                                 func=AF.Copy)
            nc.scalar.activation(out=mv[sl, 1, 0:SW], in_=ps_mvh[h][32:34, :],
                                 func=AF.Copy)
            if init_tail_ps is not None:
                nc.scalar.activation(out=mv[sl, :, SW:SP],
                                     in_=init_tail_ps[:, 4:12], func=AF.Copy)
            else:
                nc.vector.memset(mv[sl, :, SW:SP], 1.0)
            m_r, v_r = mv[sl, 0, :], mv[sl, 1, :]
            nc.vector.tensor_tensor(out=msq[sl], in0=m_r, in1=m_r, op=AL.mult)
            nc.vector.tensor_tensor(out=vv[sl], in0=v_r, in1=msq[sl],
                                    op=AL.subtract)
            nc.scalar.activation(out=vv[sl], in_=vv[sl], func=AF.Sqrt,
                                 bias=eps4[sl, 0:1])
            with nc.allow_low_precision(reason="LN rows are f16 by design"):
                nc.vector.reciprocal(out=rstd[sl], in_=vv[sl])
            return rstd[sl], m_r

        def ln_tiles():
            mv = work.tile([NL, 2, SP], F32, name="mv", tag="mvrow", bufs=2)
            msq = work.tile([NL, SP], F32, name="msq", tag="msqh", bufs=2)
            vv = work.tile([NL, SP], F32, name="vv", tag="vvh", bufs=2)
            rstd = work.tile([NL, SP], F16, name="rstd", tag="rstdh", bufs=2)
            return mv, msq, vv, rstd

        def read_tsc(ps_tsc):
            r = row4("tsc", dt=F16)
            if scb_t is None:
                nc.scalar.activation(out=r[:, 0:SW], in_=ps_tsc, func=AF.Copy)
            else:
                nc.scalar.activation(out=r[:, 0:SW], in_=ps_tsc,
                                     func=AF.Identity, bias=scb_t[:, 0:1])
            nc.vector.memset(r[:, SW:SP], 0.0)
            return r

        # ------------------------------------------------------------------
        # apply: dst[:, :, 0:512] = rA*pre - rB + rC*seq_old.
        # cols >= 512 never change: in-place dst==seqT keeps them for free.
        # init (full=True) writes the full 516 and has no rC term.
        # ------------------------------------------------------------------
        def apply_ln(b, pre, bc, dst, per_chunk_dma=None, t2_pre=None,
                     full=False):
            rAB, rBB, rCB = bc
            w = SP if full else SW
            for c in range(DC):
                t1 = work.tile([128, w], F16, name="t1g", tag="t1g", bufs=(2 if SIM else 3))
                nc.vector.tensor_tensor(out=t1, in0=rAB,
                                        in1=pre[:, c, 0:w], op=AL.mult)
                nc.vector.tensor_tensor(out=t1, in0=t1, in1=rBB, op=AL.subtract)
                if rCB is None and t2_pre is None:
                    nc.vector.tensor_copy(out=dst[b][:, c, 0:w], in_=t1)
                else:
                    if t2_pre is not None:
                        t2 = t2_pre[c]
                    else:
                        t2 = work.tile([128, w], F16, name="t2g", tag="t2g",
                                       bufs=4)
                        eng("t2").tensor_tensor(out=t2, in0=rCB,
                                                in1=seqT[b][:, c, 0:w],
                                                op=AL.mult)
                    nc.vector.tensor_tensor(out=dst[b][:, c, 0:w], in0=t1,
                                            in1=t2, op=AL.add)
                if F8_W1:
                    nc.scalar.activation(out=seqT8[b][:, c, 0:w],
                                         in_=seqT[b][:, c, 0:w],
                                         func=AF.Copy)
                if per_chunk_dma is not None:
                    per_chunk_dma(b, c, dst[b])

        # prefetched broadcast tiles for the next step, keyed per batch
        bc_next = {}
        t2_pre = {}

        # ================= initial transform ================================
        ps_tail0 = [psum.tile([2, 12], F32, name=f"ps_tail0{h}", tag="psbc",
                              bufs=2) for h in range(2)]
        pre_t = []
        for b in range(NB):
            sA = work.tile([128, DC, SP], F16, name=f"sA{b}", tag="sA", bufs=1)
            ld_dma(sA, seqT_in.ap()[b].rearrange("c p i -> p c i"))
            pre = work.tile([128, DC, SP], F16, name=f"pre{b}", tag="compT",
                            bufs=NB)
            for c in range(DC):
                ps = psmm(f"ps_pre{b}{c}")
                for (o, s) in NSPLITS:
                    for k in range(2):
                        nc.tensor.matmul(ps[:, o:o + s],
                                         itW_t[:, k, c * 128:(c + 1) * 128],
                                         sA[:, k, o:o + s],
                                         start=(k == 0), stop=(k == 1))
                if itb_t is not None:
                    nc.scalar.activation(out=pre[:, c, :], in_=ps[:, 0:SP],
                                         func=AF.Identity,
                                         bias=itb_t[:, c:c + 1])
                else:
                    nc.scalar.activation(out=pre[:, c, :], in_=ps[:, 0:SP],
                                         func=AF.Copy)
            for c in range(DC):
                mm_stat(1, b, c, pre[:, c, 0:SW],
                        init_tail=pre[:, c, SW:SP], tail_ps=ps_tail0[b // 2],
                        tail_start=(b % 2 == 0 and c == 0), tail_stop=False)
            for c in range(DC):
                sq = work.tile([128, SP], F16, name=f"sq0_{b}{c}", tag="sq",
                               bufs=2)
                eng("sq").tensor_tensor(out=sq, in0=pre[:, c, :],
                                        in1=pre[:, c, :], op=AL.mult)
                mm_stat(2, b, c, sq[:, 0:SW],
                        init_tail=sq[:, SW:SP], tail_ps=ps_tail0[b // 2],
                        tail_start=False,
                        tail_stop=(b % 2 == 1 and c == 1))
            pre_t.append(pre)

        ln0 = ln_tiles()
        rA034 = work.tile([NL, SP], F16, name="rA034", tag="rAh", bufs=4)
        rB034 = work.tile([NL, SP], F16, name="rB034", tag="rBh", bufs=4)
        for h in range(2):
            sl = slice(32 * h, 32 * h + 2)
            rstd0, m_r0 = ln_rows_half(h, ln0, init_tail_ps=ps_tail0[h])
            nc.vector.tensor_tensor(out=rA034[sl], in0=rstd0,
                                    in1=mask4[sl], op=AL.mult)
            nc.vector.tensor_tensor(out=rB034[sl], in0=rA034[sl], in1=m_r0,
                                    op=AL.mult)
        for b in range(NB):
            rAB = pe_bcast_full2(rA034, b, f"rABi{b}", "rABf")
            rBB = pe_bcast_full2(rB034, b, f"rBBi{b}", "rBBf")
            aB = bounce_bcast(act0_in.ap()[b:b + 1, :], f"aBi{b}", "aBt")
            naB = napad(f"naBi{b}", _bcast_ap(nact0_in.ap()[b:b + 1, :]))
            bc_next[b] = (rAB, rBB, None, aB, naB, None)
        for b in range(NB):
            apply_ln(b, pre_t[b], bc_next[b][:3], seqT, full=True)
        pre_t = None

        # ================= main steps =======================================
        comp_t = [None] * NB

        lc_t = [None] * NB
        lc8_t = [None] * NB
        base_t = [None] * NB
        scan_t = [None] * NB
        inter_t = [None] * NB

        def phase_a(b, s):
            """apply + baseT + lc scan chain."""
            rAB, rBB, rCB, aB, naB, ltpB = bc_next[b]
            if s > 0:
                apply_ln(b, comp_t[b], (rAB, rBB, rCB), seqT,
                         t2_pre=t2_pre.get(b))

            # ---- lc scan chain ----
            axB = ax_tile(f"axB{b}", "axB")
            for c in range(DC):
                eng("fill_seq").tensor_tensor(out=axB[:, c, 1:SP + 1], in0=aB,
                                              in1=seqT[b][:, c, :], op=AL.mult)
            lcT = work.tile([128, DC, SP], F16, name=f"lcT{b}", tag="lcT",
                            bufs=NB)
            for c in range(DC):
                scan_fwd(eng("scan_lc"), lcT[:, c, :], naB, axB[:, c])
            if F8_W1:
                lcT8 = work.tile([128, DC, SP], F8, name=f"lcT8_{b}",
                                 tag="lcT8", bufs=NB)
                for c in range(DC):
                    nc.scalar.activation(out=lcT8[:, c, :],
                                         in_=lcT[:, c, :], func=AF.Copy)
            else:
                lcT8 = lcT
            lc_t[b] = lcT
            lc8_t[b] = lcT8

        def phase_b(b, s):
            """w1 matmuls -> interT (Act reads fused over hk pairs)."""
            lcT8 = lc8_t[b]
            interT = work.tile([128, 8, SP], W2T, name=f"interT{b}",
                               tag="interT", bufs=NB)
            if fuse_w1:
                for hp in range(4):
                    ps = psmm(f"ps_w1{b}{hp}")
                    for k in range(2):
                        hk = 2 * hp + k
                        mmdr(ps[:, k * SW:(k + 1) * SW],
                             [(w1W8[0][:, :, hk * 128:(hk + 1) * 128], lcT8),
                              (w1W8[1][:, :, hk * 128:(hk + 1) * 128],
                               seqT8[b])], F8_W1)
                    gelu_act(interT[:, 2 * hp:2 * hp + 2, 0:SW], ps, None,
                             scale=w1_scale, n=2 * SW)
            else:
                for hk in range(8):
                    ps = psmm(f"ps_w1{b}{hk}")
                    mmdr(ps[:, 0:SW],
                         [(w1W8[0][:, :, hk * 128:(hk + 1) * 128], lcT8),
                          (w1W8[1][:, :, hk * 128:(hk + 1) * 128], seqT8[b])],
                         F8_W1)
                    gelu_act(interT[:, hk, 0:SW], ps[:, 0:SW],
                             w1b_t[:, hk:hk + 1], scale=w1_scale)
            inter_t[b] = interT

        def phase_c(b, s):
            """baseT + l1/r1/l2/r2 fills + scans."""
            _, _, _, aB, naB, ltpB = bc_next[b]
            # ---- baseT = seqT + tf ----
            baseT = work.tile([128, DC, SP], CVT, name=f"baseT{b}",
                              tag="baseT", bufs=2)
            if s == 0:
                for c in range(DC):
                    nc.vector.tensor_scalar(out=baseT[:, c, :],
                                            in0=seqT[b][:, c, :],
                                            scalar1=noc[:, c:c + 1],
                                            scalar2=None, op0=AL.add)
            else:
                for c in range(DC):
                    tfB = work.tile([128, SP], F16, name=f"tfB{b}{c}",
                                    tag="tfB", bufs=(2 if SIM else 4))
                    nc.vector.tensor_scalar(out=tfB, in0=ltpB,
                                            scalar1=ymnc[:, c:c + 1],
                                            scalar2=noc[:, c:c + 1],
                                            op0=AL.mult, op1=AL.add)
                    nc.vector.tensor_tensor(out=baseT[:, c, :], in0=tfB,
                                            in1=seqT[b][:, c, :], op=AL.add)
            base_t[b] = baseT
            axb = ax_tile(f"axb{b}", "axs")
            for c in range(DC):
                eng("fill_base").tensor_tensor(out=axb[:, c, 1:SP + 1], in0=aB,
                                               in1=baseT[:, c, :], op=AL.mult)
            l1T = work.tile([128, DC, SP], CVT, name=f"l1T{b}", tag="l1T", bufs=2)
            r1T = work.tile([128, DC, SP], CVT, name=f"r1T{b}", tag="r1T", bufs=(1 if os.environ.get("CRVNN_TESTSLIM") else 2))
            for c in range(DC):
                scan_fwd(eng("scan_l1"), l1T[:, c, :], naB, axb[:, c])
                scan_bwd(eng("scan_r1"), r1T[:, c, :], naB, axb[:, c])
            ax2b = ax_tile(f"ax2b{b}", "axs")
            for c in range(DC):
                eng("fill_r2").tensor_tensor(out=ax2b[:, c, 1:SP + 1], in0=aB,
                                             in1=r1T[:, c, :], op=AL.mult)
            r2T = work.tile([128, DC, SP], CV2, name=f"r2T{b}", tag="r2T", bufs=(1 if os.environ.get("CRVNN_TESTSLIM") else 2))
            for c in range(DC):
                scan_bwd(eng("scan_r2"), r2T[:, c, :], naB, ax2b[:, c])
            ax2 = ax_tile(f"ax2{b}", "axs")
            for c in range(DC):
                eng("fill_l2").tensor_tensor(out=ax2[:, c, 1:SP + 1], in0=aB,
                                             in1=l1T[:, c, :], op=AL.mult)
            l2T = work.tile([128, DC, SP], CV2, name=f"l2T{b}", tag="l2T", bufs=(1 if os.environ.get("CRVNN_TESTSLIM") else 2))
            for c in range(DC):
                scan_fwd(eng("scan_l2"), l2T[:, c, :], naB, ax2[:, c])
            scan_t[b] = (l1T, r1T, l2T, r2T)

        def phase_dc(b, s, ps_tsc):
            """conv -> gT -> tsc accumulate (one fused gelu per batch)."""
            baseT = base_t[b]
            l1T, r1T, l2T, r2T = scan_t[b]
            pieces = [(2, baseT, F8_CV), (3, r1T, F8_CV), (1, l1T, F8_CV),
                      (4, r2T, F8_CV or MIXED_CV), (0, l2T, F8_CV or MIXED_CV)]
            ps = psmm(f"ps_cv{b}")
            for c in range(DC):
                K = len(pieces)
                o, sw_ = c * SW, SW
                for k, (w, piece, pf8) in enumerate(pieces):
                    if MIXED_CV and pf8:
                        lhsT = cvW8f8[0 if w == 0 else 1][:, :, c * 128:(c + 1) * 128]
                    else:
                        lhsT = cvW8[w][:, :, c * 128:(c + 1) * 128]
                    if pf8:
                        nc.tensor.matmul(ps[:, o:o + sw_], lhsT,
                                         piece[:, :, 0:SW],
                                         start=(k == 0), stop=(k == K - 1),
                                         perf_mode=PM.DoubleRow)
                    else:
                        for i in range(2):
                            nc.tensor.matmul(ps[:, o:o + sw_], lhsT[:, i, :],
                                             piece[:, i, 0:SW],
                                             start=(k == 0 and i == 0),
                                             stop=(k == K - 1 and i == 1))
            if fuse_cv:
                gT = work.tile([128, DC, SW], F16, name=f"gT{b}", tag="gT",
                               bufs=2)
                gelu_act(gT, ps, None, scale=cv_scale, n=2 * SW)
            else:
                gT = work.tile([128, DC, SW], F16, name=f"gT{b}", tag="gT",
                               bufs=2)
                for c in range(DC):
                    gelu_act(gT[:, c, :], ps[:, c * SW:(c + 1) * SW],
                             convb_t[:, c:c + 1], scale=cv_scale)
            for c in range(DC):
                nc.tensor.matmul(ps_tsc, sc4[:, b, c, :], gT[:, c, :],
                                 start=(b == 0 and c == 0),
                                 stop=(b == NB - 1 and c == 1))

        def phase_dw(b, s):
            """w2 -> gates/parent -> comp -> LN stats."""
            lcT = lc_t[b]
            interT = inter_t[b]
            # gate order: parent first, then g2 so its Pool multiply (m2)
            # overlaps the remaining gate reads; the rest accumulate on DVE.
            comp = work.tile([128, DC, SW], F16, name=f"comp{b}", tag="compT",
                             bufs=NB)
            parT = work.tile([128, DC, SW], F16, name=f"parT{b}", tag="gpar",
                             bufs=2)
            m2_t = [None] * DC
            m1_t = [None] * DC
            GORD = [int(x) for x in os.environ.get("CRVNN_GORD", "3201")]
            for g in GORD:
                ps = psmm(f"ps_w2{b}{g}")
                for c in range(DC):
                    cc = g * DC + c
                    mmdr(ps[:, c * SW:(c + 1) * SW],
                         [(w2W8[p][:, :, cc * 128:(cc + 1) * 128],
                           interT[:, 2 * p:2 * p + 2, :]) for p in range(4)],
                         F8_W2)
                if g == 3:
                    if fuse_w2:
                        nc.scalar.activation(out=parT, in_=ps, func=AF.Identity,
                                             scale=w2_scale)
                    else:
                        for c in range(DC):
                            nc.scalar.activation(
                                out=parT[:, c, :], in_=ps[:, c * SW:(c + 1) * SW],
                                func=AF.Identity, bias=w2b_t[:, 2 * g + c + 0:2 * g + c + 1],
                                scale=w2_scale)
                else:
                    gate2 = work.tile([128, DC, SW], F16, name=f"gate{b}{g}",
                                      tag="gate", bufs=4)
                    if fuse_w2:
                        nc.scalar.activation(out=gate2, in_=ps, func=AF.Sigmoid,
                                             scale=w2_scale)
                    else:
                        for c in range(DC):
                            cc = g * DC + c
                            nc.scalar.activation(
                                out=gate2[:, c, :], in_=ps[:, c * SW:(c + 1) * SW],
                                func=AF.Sigmoid, bias=w2b_t[:, cc:cc + 1],
                                scale=w2_scale)
                    for c in range(DC):
                        if g == 2:
                            m2 = work.tile([128, SW], F16, name=f"gm2_{b}",
                                           tag="gmt", bufs=(2 if SIM else 4))
                            nc.gpsimd.tensor_tensor(out=m2, in0=gate2[:, c, :],
                                                    in1=parT[:, c, :],
                                                    op=AL.mult)
                            m2_t[c] = m2
                        elif g == 1:
                            m1 = work.tile([128, SW], F16, name=f"gm1_{b}",
                                           tag="gm1t", bufs=(2 if SIM else 4))
                            eng("m1").tensor_tensor(out=m1, in0=gate2[:, c, :],
                                                    in1=seqT[b][:, c, 0:SW],
                                                    op=AL.mult)
                            m1_t[c] = m1
                        else:
                            eng("cmul").tensor_tensor(out=comp[:, c, :],
                                                      in0=gate2[:, c, :],
                                                      in1=lcT[:, c, 0:SW],
                                                      op=AL.mult)
            for c in range(DC):
                nc.vector.tensor_tensor(out=comp[:, c, :], in0=comp[:, c, :],
                                        in1=m1_t[c], op=AL.add)
                nc.vector.tensor_tensor(out=comp[:, c, :], in0=comp[:, c, :],
                                        in1=m2_t[c], op=AL.add)
            comp_t[b] = comp

            # ---- LN stats of comp ----
            for c in range(DC):
                mm_stat(1, b, c, comp[:, c, :])
            for c in range(DC):
                sq = work.tile([128, SW], F16, name=f"sq{b}{c}", tag="sq",
                               bufs=2)
                eng("sq").tensor_tensor(out=sq, in0=comp[:, c, :],
                                        in1=comp[:, c, :], op=AL.mult)
                mm_stat(2, b, c, sq)

        def tail_tp(s, ps_tsc):
            if "t" in HIPRI:
                with tc.high_priority():
                    return _tail_tp(s, ps_tsc)
            return _tail_tp(s, ps_tsc)

        def _tail_tp(s, ps_tsc):
            """tp/active rows; needs only the tsc stats (conv phase) -> runs
            concurrently with the w2 phase."""
            last = (s == n_steps - 1)
            if not last:
                asq = row4("asq")
                (G if ENG.get("act_upd", "g") == "g" else V).tensor_tensor(
                    out=asq, in0=a4, in1=a4, op=AL.mult)

            # tp = selp * sigmoid(tsc): the reference's
            # tp = e^{t-mx}selp/(e^{t-mx}selp + e^{-mx} + EPS) equals this up
            # to O(EPS); scores are O(1) so no overflow concern.
            tsc = read_tsc(ps_tsc)
            sig = row4("sig", dt=F16)
            nc.scalar.activation(out=sig, in_=tsc, func=AF.Sigmoid)
            tpp = work.tile([NL, SPP], F16, name="tpp", tag="rowP", bufs=3)
            nc.vector.memset(tpp[:, 0:SPP:SPP - 1], 0.0)
            tp = tpp[:, 1:SP + 1]
            nc.vector.tensor_tensor(out=tp, in0=sig, in1=selp4, op=AL.mult)
            tpm = row4("tpm", dt=F16)
            nc.vector.tensor_tensor(out=tpm, in0=tp, in1=mask4, op=AL.mult)
            rC = row4("rC", dt=F16)
            nc.vector.tensor_tensor(out=rC, in0=mask4, in1=tpm, op=AL.subtract)

            if last:
                return tpm, rC, None, None, None

            # active update: a_new = clip(a - a^2*u, 0, 1)*mask.  This
            # branch only feeds next-step rows that travel via slack DMA
            # broadcasts, so it can run on the boundary-idle engine (AU).
            AU = G if ENG.get("act_upd", "g") == "g" else V
            nap = work.tile([NL, SPP], F16, name="nap", tag="rowP", bufs=3)
            nc.vector.memset(nap[:, 0:SPP:SPP - 1], 0.0)
            nc.vector.tensor_scalar(out=nap[:, 1:SP + 1], in0=a4,
                                    scalar1=-1.0, scalar2=1.0,
                                    op0=AL.mult, op1=AL.add)
            u = row4("u")
            nc.vector.tensor_tensor_scan(
                out=u[:, ::-1], data0=nap[:, SPP - 1:1:-1],
                data1=tpp[:, SPP - 1:1:-1], initial=0.0,
                op0=AL.mult, op1=AL.add)
            nd = row4("nd")
            AU.tensor_tensor(out=nd, in0=asq, in1=u, op=AL.mult)
            AU.tensor_tensor(out=nd, in0=a4, in1=nd, op=AL.subtract)
            nc.vector.tensor_scalar(out=nd, in0=nd, scalar1=0.0,
                                    scalar2=1.0, op0=AL.max, op1=AL.min)
            AU.tensor_tensor(out=a4, in0=nd, in1=mask4, op=AL.mult)
            a_f = row4("a_f", dt=F16)
            nc.vector.tensor_scalar(out=a_f, in0=a4, scalar1=1.0,
                                    scalar2=None, op0=AL.mult)
            na_f = row4("na_f", dt=F16)
            nc.vector.tensor_scalar(out=na_f, in0=a4, scalar1=-1.0,
                                    scalar2=1.0, op0=AL.mult, op1=AL.add)
            return tpm, rC, a_f, na_f, tp

        def dma_bcast(row_sb, s, name, tag, width=SP, pads=False, w0=0,
                      bufs=4):
            """Bounce a (4,*) SBUF row tile through DRAM and broadcast each
            batch row to (128,*) via partition-broadcast DMA reads.  All
            engine-free (HWDGE); latency ~3-4us, fine for next-phase rows."""
            rd = dram.tile([NB, width], F16, name=f"d_{name}{s}",
                           tag=f"d_{tag}", bufs=2)
            nc.sync.dma_start(out=rd[0:2, :], in_=row_sb[0:2, w0:w0 + width])
            nc.sync.dma_start(out=rd[2:4, :],
                              in_=row_sb[32:34, w0:w0 + width])
            outs = []
            for b in range(NB):
                if pads:
                    t = work.tile([128, SPP], F16, name=f"{name}{b}",
                                  tag="nabP", bufs=3)
                    nc.vector.memset(t[:, 0:SPP:SPP - 1], 0.0)
                    nc.sync.dma_start(out=t[:, 1:SP + 1],
                                        in_=_bcast_ap(rd[b:b + 1, :]))
                else:
                    t = bc_tile(f"{name}{b}", tag=tag, bufs=bufs, w=width)
                    nc.sync.dma_start(out=t, in_=_bcast_ap(rd[b:b + 1, :]))
                outs.append(t)
            return outs

        def tail_bc(s, rows):
            """row broadcasts at the step boundary via DMA bounce (engine-
            free).  Also precompute the apply's rC*seq_old terms here."""
            tpm, rC, a_f, na_f, tp = rows
            rCBs = dma_bcast(rC, s, f"rCBs{s}", "rCBt", width=SW, bufs=3)
            if a_f is not None:
                aBs = dma_bcast(a_f, s, f"aBs{s}", "aBt")
                naBs = dma_bcast(na_f, s, f"naBs{s}", None, pads=True)
                ltpBs = dma_bcast(tp, s, f"ltpBs{s}", "ltpBt")
            for b in range(NB):
                t2s = []
                for c in range(DC):
                    t2 = work.tile([128, SW], F16, name=f"t2e{b}{c}",
                                   tag="t2e", bufs=2 * NB)
                    eng("t2").tensor_tensor(out=t2, in0=rCBs[b],
                                            in1=seqT[b][:, c, 0:SW],
                                            op=AL.mult)
                    t2s.append(t2)
                t2_pre[b] = t2s
                if a_f is None:
                    bc_next[b] = ()
                else:
                    bc_next[b] = (aBs[b], naBs[b], ltpBs[b])

        BCE = os.environ.get("CRVNN_BCE", "aa")
        ln_state = {}

        HIPRI = os.environ.get("CRVNN_HIPRI", "0")

        def tail_ln_half(s, h, tpm):
            """LN rows for batch pair {2h,2h+1} of step s.  Called at the
            head of step s+1 so half 1's chain never head-of-line-blocks
            half 0's apply in the DVE queue."""
            if "l" in HIPRI:
                with tc.high_priority():
                    _tail_ln_half(s, h, tpm)
            else:
                _tail_ln_half(s, h, tpm)

        def _tail_ln_half(s, h, tpm):
            if h == 0:
                ln_state[s] = ln_tiles() + (
                    work.tile([NL, SP], F16, name=f"rA{s}", tag="rAh", bufs=4),
                    work.tile([NL, SP], F16, name=f"rB{s}", tag="rBh", bufs=4))
            tiles = ln_state[s]
            rA, rB = tiles[4], tiles[5]
            sl = slice(32 * h, 32 * h + 2)
            rstd_h, m_r = ln_rows_half(h, tiles[:4])
            nc.vector.tensor_tensor(out=rA[sl], in0=tpm[sl],
                                    in1=rstd_h, op=AL.mult)
            nc.vector.tensor_tensor(out=rB[sl], in0=rA[sl], in1=m_r,
                                    op=AL.mult)
            for b in (2 * h, 2 * h + 1):
                rAB = pe_bcast2(rA, b, f"rABs{s}{b}", "rABt",
                                {"d": "dve", "p": "act", "a": "act"}[BCE[0]])
                rBB = pe_bcast2(rB, b, f"rBBs{s}{b}", "rBBt",
                                {"d": "dve", "p": "act", "a": "act"}[BCE[1]])
                bc_next[b] = (rAB, rBB, None) + bc_next[b]

        last_rows = None
        IL = int(os.environ.get("CRVNN_IL", "0"))
        for s in range(n_steps):
            if IL == 0:
                for h in range(2):
                    if s > 0:
                        tail_ln_half(s - 1, h, last_rows[0])
                for b in range(NB):
                    phase_a(b, s)
                for b in range(NB):
                    phase_b(b, s)
            elif IL == 1:
                for h in range(2):
                    if s > 0:
                        tail_ln_half(s - 1, h, last_rows[0])
                    for b in (2 * h, 2 * h + 1):
                        phase_a(b, s)
                for b in range(NB):
                    phase_b(b, s)
            else:
                for h in range(2):
                    if s > 0:
                        tail_ln_half(s - 1, h, last_rows[0])
                    for b in (2 * h, 2 * h + 1):
                        phase_a(b, s)
                    for b in (2 * h, 2 * h + 1):
                        phase_b(b, s)
            ps_tsc = psum.tile([NL, SW], F32, name=f"ps_tsc{s}", tag="psbc",
                               bufs=2)
            for b in range(NB):
                phase_c(b, s)
                phase_dc(b, s, ps_tsc)
            TPPOS = int(os.environ.get("CRVNN_TPPOS", "0"))
            rows = None
            if TPPOS == 0:
                rows = tail_tp(s, ps_tsc)
            for b in range(NB):
                phase_dw(b, s)
                if b + 1 == TPPOS:
                    rows = tail_tp(s, ps_tsc)
            if rows is None:
                rows = tail_tp(s, ps_tsc)
            tail_bc(s, rows)
            last_rows = rows

        # ---------------- final apply (f32 out) + DMA ----------------------
        # cols [512:514) never changed: copy them from seqT into the f32 out.
        outF = [work.tile([128, DC, SP], F16, name=f"outF{b}", tag="outF",
                          bufs=2) for b in range(NB)]

        def out_dma(b, c, dst):
            nc.vector.tensor_copy(out=dst[:, c, SW:S2],
                                  in_=seqT[b][:, c, SW:S2])
            nc.sync.dma_start(out=out_dram.ap()[b, c], in_=dst[:, c, 0:S2])

        for h in range(2):
            tail_ln_half(n_steps - 1, h, last_rows[0])
            for b in (2 * h, 2 * h + 1):
                apply_ln(b, comp_t[b], bc_next[b][:3], outF,
                         per_chunk_dma=out_dma, t2_pre=t2_pre.get(b))
    return nc


def _host_prep(inputs):
    f32 = np.float32
    f16 = np.float16
    f8 = ml_dtypes.float8_e4m3
    seq = np.asarray(inputs["sequence"], f32)
    im = np.asarray(inputs["input_mask"], f32)
    START = np.asarray(inputs["START"], f32)
    END = np.asarray(inputs["END"], f32)
    yes_t = np.asarray(inputs["yes_t"], f32).reshape(-1)
    no_t = np.asarray(inputs["no_t"], f32).reshape(-1)
    N, S, Dd = seq.shape
    assert (N, S, Dd) == (32, 512, 256), (N, S, Dd)

    ones = np.ones((N, 1, 1), f32)
    zeros = np.zeros((N, 1, 1), f32)
    mask0 = np.concatenate([ones, im], 1)
    mask_no_end = np.concatenate([mask0, zeros], 1)
    mask_yes_end = np.concatenate([ones, mask0], 1)
    END_mask = mask_yes_end - mask_no_end
    seqA = np.concatenate([np.broadcast_to(START, (N, 1, Dd)), seq,
                           np.zeros((N, 1, Dd), f32)], 1)
    seqA = (END_mask * END + (1.0 - END_mask) * seqA).astype(f32)
    mask = mask_yes_end
    mask_no_start = np.concatenate([zeros, mask[:, 1:]], 1)
    last_tok = np.concatenate([END_mask[:, 1:], zeros], 1)
    selp = (mask_no_start * mask_no_end * (1.0 - last_tok)).astype(f32)

    seqT = np.zeros((N, DC, 128, SP), f32)
    for c in range(DC):
        seqT[:, c, :, :S2] = seqA[:, :, c * 128:(c + 1) * 128].transpose(0, 2, 1)
    maskP = np.zeros((N, SP), f32)
    maskP[:, :S2] = mask[:, :, 0]
    selpP = np.zeros((N, SP), f32)
    selpP[:, :S2] = selp[:, :, 0]
    actP = maskP.copy()
    nactP = (1.0 - actP).astype(f32)

    # 34-lane row layout: batch b of a core's 4 -> partition b%2 + 32*(b//2)
    LANES = [0, 1, 32, 33]

    def lanes34(rows):  # (N, SP) -> per-core (N//NB, 34, SP) lane layout
        out = np.zeros((N // NB, 34, rows.shape[1]), rows.dtype)
        per = rows.reshape(N // NB, NB, -1)
        for b in range(NB):
            out[:, LANES[b], :] = per[:, b, :]
        return out

    def chunk_col(v, nch):
        return np.ascontiguousarray(np.asarray(v, f32).reshape(nch, 128).T)

    scW = np.asarray(inputs["scW"], f32).reshape(-1)
    sc4 = np.zeros((128, NB, DC, 34), f32)
    for b in range(NB):
        for c in range(DC):
            sc4[:, b, c, LANES[b]] = scW[c * 128:(c + 1) * 128]
    obD = np.zeros((128, NB, 2), f32)
    for b in range(NB):
        obD[:, b, b % 2] = 1.0 / D
    bsel = np.zeros((34, NB, 128), f32)
    for b in range(NB):
        bsel[LANES[b], b, :] = 1.0

    def wconv(name, use_f8):
        w = np.asarray(inputs[name], f32)
        return (w * WSCALE).astype(f8) if use_f8 else w.astype(f16)

    def wconv_mixed(name):
        # uniform x64 prescale so f16 and f8 pieces share one PSUM-read
        # scale; returns the f16 (all pieces) tensor.
        w = np.asarray(inputs[name], f32)
        return (w * WSCALE).astype(f16)

    host = {
        "seqT": seqT.astype(f16),
        "mask": lanes34(maskP).astype(f16),
        "selp": lanes34(selpP).astype(f16),
        "act0": actP.astype(f16),
        "act0f": lanes34(actP),
        "nact0": nactP.astype(f16),
        "itW": np.asarray(inputs["itW"], f32).astype(f16),
        "convW": wconv_mixed("convW") if MIXED_CV else wconv("convW", F8_CV),
        "convW8": np.concatenate(
            [np.asarray(inputs["convW"], f32)[0 * D:1 * D],
             np.asarray(inputs["convW"], f32)[4 * D:5 * D]],
            0).astype(f8) * 1 if False else (np.concatenate(
                [np.asarray(inputs["convW"], f32)[0 * D:1 * D],
                 np.asarray(inputs["convW"], f32)[4 * D:5 * D]], 0)
                * WSCALE).astype(f8),
        "w1W": wconv("w1W", F8_W1),
        "w2W": wconv("w2W", F8_W2),
        "sc4": sc4.astype(f16),
        "obD": obD.astype(f16),
        "bsel": bsel.astype(f16),
        "noc": chunk_col(no_t, DC),
        "ymnc": chunk_col(yes_t - no_t, DC),
    }
    flags = {
        "itbc": bool(np.any(np.asarray(inputs["itb"]))),
        "convbc": bool(np.any(np.asarray(inputs["convb"]))),
        "w1bc": bool(np.any(np.asarray(inputs["w1b"]))),
        "w2bc": bool(np.any(np.asarray(inputs["w2b"]))),
        "scbc": bool(np.any(np.asarray(inputs["scb"]))),
        "lngc": bool(np.any(np.asarray(inputs["lnb"])))
        or bool(np.any(np.asarray(inputs["lng"]) != 1.0)),
    }
    flags["lnbc"] = flags["lngc"]
    assert not flags["lngc"], "v4 kernel assumes identity LN gain/bias"
    if flags["itbc"]:
        host["itbc"] = chunk_col(inputs["itb"], DC)
    if flags["convbc"]:
        host["convbc"] = chunk_col(inputs["convb"], DC)
    if flags["w1bc"]:
        host["w1bc"] = chunk_col(inputs["w1b"], 8)
    if flags["w2bc"]:
        host["w2bc"] = chunk_col(inputs["w2b"], 8)
    if flags["scbc"]:
        host["scbc"] = np.broadcast_to(
            np.asarray(inputs["scb"], f32).reshape(1, 1), (4, 1)).copy()
    return host, flags


_PROG_CACHE = {}


def kernel(**inputs):
    global LAST_EXEC_NS, LAST_RES
    n_steps = int(inputs["n_steps"])
    host, flags = _host_prep(inputs)

    key = (n_steps, tuple(sorted(flags.items())), MM_DT, W2_DT, SIM, GP_LVL,
           F8_W1, F8_W2, F8_CV)
    if key not in _PROG_CACHE:
        _PROG_CACHE[key] = _build_program(n_steps, flags)
    nc = _PROG_CACHE[key]

    per_batch = {"seqT", "act0", "nact0"}
    per_core = {"mask", "selp", "act0f"}
    in_maps = []
    for k in range(NCORES):
        m = {}
        for name, arr in host.items():
            if name in per_batch:
                m[name] = np.ascontiguousarray(arr[k * NB:(k + 1) * NB])
            elif name in per_core:
                m[name] = np.ascontiguousarray(arr[k])
            else:
                m[name] = arr
        in_maps.append(m)

    if SIM:
        from concourse.bass_interp import CoreSim
        results = []
        for k in range(int(os.environ.get("CRVNN_SIM_CORES", "1"))):
            sim = CoreSim(nc)
            for name, v in in_maps[k].items():
                sim.tensor(name)[:] = v
            sim.simulate()
            results.append(np.array(sim.tensor("out")))
    else:
        from concourse.bass_utils import run_bass_kernel_spmd
        if not getattr(nc, "_waitfix_done", False):
            _split_multiwaits(nc)
            nc._waitfix_done = True
        res = run_bass_kernel_spmd(nc, in_maps, list(range(NCORES)), trace=TRACE)
        LAST_EXEC_NS = res.exec_time_ns
        LAST_RES = res
        results = [res.results[k]["out"] for k in range(NCORES)]

    full = np.zeros((32, S2, D), np.float32)
    for k, o in enumerate(results):
        for b in range(NB):
            for c in range(DC):
                full[k * NB + b, :, c * 128:(c + 1) * 128] = \
                    np.asarray(o[b, c], np.float32).T
    return full
